# revision 1
# baseline (speedup 1.0000x reference)
"""HMM posterior kernel for Trainium2 (8 NeuronCores, SPMD data-parallel over batch).

Math: in the reference,
    ln_fs + ln_bs = 2*ln_pi + ln_emis[:,T-1,:] + total + (T-1)*ln_diag
(the cumsum terms cancel), so the pre-normalization log_gamma is independent
of t and the output is a [B, K] tensor broadcast over T.  With
    S1[b] = sum_t x, S2[b] = sum_t x^2, xl[b] = x[T-1],
    u = S2 + xl^2, v = S1 + xl, P' = exp(-2*ls),
the pre-norm value is rank-2 in the batch:
    g[b,k] = P'[k]*(-u[b]/2) + Q[k]*v[b] + R[k]
    Q = P'*mu
    R = -0.5*(T+1)*P'*mu^2 - (T+1)*ls + 2*pi + (T-1)*di
(the -(T+1)*C constant cancels in the normalization), and
out[b,t,:] = g[b,:] - logsumexp_k g[b,:] for every t.

Each core handles B/8 = 4 batch rows.  The param/g chain runs on an
[8, 256] layout (partition r = khalf*4 + b, 256 k's per partition) so each
fp16 DVE op moves half the per-partition elements; the host pre-scales the
tiny param vectors (cmu = -0.5*(T+1)*mu, p2 = 2*pi, dT = (T-1)*di,
lT = -(T+1)*ls) so the R chain is pure fp16 tensor_tensor (fast 16-bit
path).  Obvs stats: one fused DVE reduce + PE ones/e127 contraction (xl
folded in, -1/2 folded into the transpose matmul's rhs scale); the
[1, 2BS] stat row is replicated to [1, 4BS] by a stride-0 ACT copy so the
transpose matmuls emit [8, 1] per-partition scalars directly.  The
cross-half logsumexp combine runs on DVE: a stream_shuffle swaps
partition r with r^4, then ALU min (for the max) / add (for the sums).  g8 is broadcast to [128, 512] PSUM by per-half
PE fp16 selector matmuls; delta = -max - ln(sum) is PE-broadcast into a
PSUM column block and applied inside the PSUM->SBUF copies (whole rows
alternate DVE tensor_scalar / ACT Identity-with-bias), which pack TWO fp16
copies of the row per partition so the output DMA moves 2 KB lines (fp16
halves HBM write traffic; the host upcasts to f32; scale-relative error
~1.5e-3 vs the 2e-2 gate).  The kernel is output-write bound (memory
regime).
"""

import numpy as np

B, T, K = 32, 2048, 512
NCORES = 8
BS = B // NCORES  # 4 batch rows per core
W = 16            # t = p*W + w layout for the obvs stats pass
KH = K // 2       # 256 k's per partition in the [8, KH] chain layout
RJ2 = T // 256    # 8 stride-0 repeats of a [128, 2K] fp16 tile per batch row
LOG_2PI = float(np.log(2.0 * np.pi))
C = 0.5 * LOG_2PI
LN2 = float(np.log(2.0))

_BUILT = {}


def _const_misc() -> np.ndarray:
    # [128, 27] f32: col0 = ones (partition contraction), col1 = e127
    # (selects the t=T-1 column), col2 = [-0.5, 0, ...] (scale for the
    # u-transpose matmul; only [0,2] is read); cols 3:27 are unused
    # combine matrices kept for layout stability.
    m = np.zeros((128, 27), dtype=np.float32)
    m[:, 0] = 1.0
    m[127, 1] = 1.0
    m[0, 2] = -0.5
    for j in range(8):
        for c in range(8):
            if j == c:
                m[j, 3 + c] = 0.5
                m[j, 11 + c] = 0.5
                m[j, 19 + c] = 1.0
            elif j == c ^ 4:
                m[j, 3 + c] = -0.5
                m[j, 11 + c] = 0.5
                m[j, 19 + c] = 1.0
    return m


def _const_sel() -> np.ndarray:
    # [8, 8*128] fp16: sel[:, i*128:(i+1)*128] = e_i (x) ones[128];
    # lhsT of the PE matmuls replicating chain row i across 128 partitions.
    s = np.zeros((8, 8 * 128), dtype=np.float16)
    for i in range(8):
        s[i, i * 128 : (i + 1) * 128] = 1.0
    return s


def _build_nc(split_waits=True):
    key = ("nc", split_waits)
    if key in _BUILT:
        return _BUILT[key]

    from concourse import bass, tile
    import concourse.mybir as mybir

    f32 = mybir.dt.float32
    f16 = mybir.dt.float16
    AF = mybir.ActivationFunctionType
    ALU = mybir.AluOpType
    X = mybir.AxisListType.X

    nc = bass.Bass()
    obvs = nc.declare_dram_parameter("obvs", [BS, T], f32, isOutput=False)
    prm16 = nc.declare_dram_parameter("prm16", [6, 8, KH], f16, isOutput=False)
    c_misc = nc.declare_dram_parameter("c_misc", [128, 27], f32, isOutput=False)
    c_sel = nc.declare_dram_parameter("c_sel", [8, 8 * 128], f16, isOutput=False)
    out = nc.declare_dram_parameter("out", [BS, T, K], f16, isOutput=True)

    with tile.TileContext(nc) as tc:
        with (
            tc.tile_pool(name="sbuf", bufs=1) as pool,
            tc.tile_pool(name="psum", bufs=1, space="PSUM") as psum,
        ):
            # ---- loads: sync: obvs, mu, lT; gpsimd: ls, misc, cmu, p2,
            # dT, sel; scalar queue stays free so the ACT table loads
            # immediately.
            cmb = pool.tile([128, 2, BS, W], f32)
            nc.sync.dma_start(
                out=cmb[:, 0], in_=obvs[:].rearrange("b (p w) -> p b w", w=W)
            )
            # ALL six pre-scaled param rows land in ONE DMA (the serial
            # per-DMA descriptor-gen cost on the queue was gating the chain)
            pall = pool.tile([8, 6, KH], f16)
            nc.gpsimd.dma_start(
                out=pall[:], in_=prm16[:].rearrange("w r k -> r w k")
            )
            misc = pool.tile([128, 27], f32)
            nc.gpsimd.dma_start(out=misc[:], in_=c_misc[:])
            sel8 = pool.tile([8, 8 * 128], f16)
            nc.gpsimd.dma_start(out=sel8[:], in_=c_sel[:])
            mu8 = pall[:, 0]
            ls8 = pall[:, 1]
            cmu8 = pall[:, 2]
            p28 = pall[:, 3]
            dT8 = pall[:, 4]
            lT8 = pall[:, 5]
            ones_col = misc[:, 0:1]
            e127_col = misc[:, 1:2]
            one_s = misc[0:1, 0:1]
            neghalf_s = misc[0:1, 2:3]
            mdiff = misc[0:8, 3:11]
            msum = misc[0:8, 11:19]
            msum1 = misc[0:8, 19:27]

            # ---- obvs stats: x^2 alongside x, one fused reduce (DVE) ----
            nc.vector.tensor_mul(cmb[:, 1], cmb[:, 0], cmb[:, 0])
            sp = pool.tile([128, 2, BS], f32)
            nc.vector.reduce_sum(sp[:].unsqueeze(3), cmb[:], axis=X)

            # ---- param chain on [8, KH]: P' on ACT, R chain fp16 tt ----
            P8 = pool.tile([8, KH], f16)
            nc.scalar.activation(P8[:], ls8, AF.Exp, scale=-2.0)
            Q8 = pool.tile([8, KH], f16)
            nc.vector.tensor_mul(Q8[:], P8[:], mu8)
            mm2 = pool.tile([8, KH], f16)
            nc.vector.tensor_mul(mm2[:], Q8[:], cmu8)
            s1 = pool.tile([8, KH], f16)
            nc.vector.tensor_add(s1[:], lT8, mm2[:])
            s2 = pool.tile([8, KH], f16)
            nc.vector.tensor_add(s2[:], p28, s1[:])
            R8 = pool.tile([8, KH], f16)
            nc.vector.tensor_add(R8[:], dT8, s2[:])

            # ---- PE contraction: ps_s[0, :] = [v-block | u-block] ----
            ps_s = psum.tile([1, 2 * BS], f32)
            nc.tensor.matmul(
                ps_s[:],
                lhsT=ones_col,
                rhs=sp[:].rearrange("p a b -> p (a b)"),
                start=True,
                stop=False,
            )
            nc.tensor.matmul(
                ps_s[:, 0:BS],
                lhsT=e127_col,
                rhs=cmb[:, 0, :, W - 1],
                start=False,
                stop=False,
                skip_group_check=True,
            )
            nc.tensor.matmul(
                ps_s[:, BS : 2 * BS],
                lhsT=e127_col,
                rhs=cmb[:, 1, :, W - 1],
                start=False,
                stop=True,
                skip_group_check=True,
            )
            # replicate [v | u] to [v v u u] so the transpose matmuls can
            # emit [8, 1] per-partition scalars (rows b and b+4 equal).
            srow2 = pool.tile([1, 2, 2, BS], f32)
            nc.scalar.copy(
                srow2[0:1, 0],
                ps_s[0:1, 0:BS].unsqueeze(1).broadcast_to([1, 2, BS]),
            )
            nc.scalar.copy(
                srow2[0:1, 1],
                ps_s[0:1, BS : 2 * BS].unsqueeze(1).broadcast_to([1, 2, BS]),
            )
            v_lhsT = srow2[0:1, 0].rearrange("a c b -> a (c b)")
            u_lhsT = srow2[0:1, 1].rearrange("a c b -> a (c b)")
            ps_t = psum.tile([8, 2], f32)
            nc.tensor.matmul(
                ps_t[:, 0:1], lhsT=v_lhsT, rhs=one_s, start=True, stop=True
            )
            nc.tensor.matmul(
                ps_t[:, 1:2], lhsT=u_lhsT, rhs=neghalf_s, start=True, stop=True
            )
            v_col = ps_t[:, 0:1]
            uneg_col = ps_t[:, 1:2]

            # ---- g = P'*(-u/2) + Q*v + R  (two fused DVE ops, fp16) ----
            g1 = pool.tile([8, KH], f16)
            nc.vector.scalar_tensor_tensor(
                out=g1[:], in0=P8[:], scalar=uneg_col, in1=R8[:],
                op0=ALU.mult, op1=ALU.add,
            )
            g8 = pool.tile([8, KH], f16)
            nc.vector.scalar_tensor_tensor(
                out=g8[:], in0=Q8[:], scalar=v_col, in1=g1[:],
                op0=ALU.mult, op1=ALU.add,
            )

            # ---- logsumexp across both halves of each row ----
            negm8 = pool.tile([8, 1], f32)
            nc.vector.reduce_max(negm8[:], g8[:], axis=X, negate=True)
            # combine across half-pairs on DVE: a stream_shuffle swaps
            # partition r with r^4 (symmetric mask, so either shuffle
            # semantics works), then ALU min / add.
            swap = [4, 5, 6, 7, 0, 1, 2, 3] + list(range(8, 32))
            shm = pool.tile([8, 1], f32)
            nc.vector.stream_shuffle(shm[:], negm8[:], swap)
            negmC = pool.tile([8, 1], f32)
            nc.vector.tensor_tensor(negmC[:], negm8[:], shm[:], op=ALU.min)
            e8 = pool.tile([8, KH], f32)
            s8 = pool.tile([8, 1], f32)
            nc.scalar.activation(
                e8[:], g8[:], AF.Exp, bias=negmC[:], accum_out=s8[:]
            )
            shs = pool.tile([8, 1], f32)
            nc.vector.stream_shuffle(shs[:], s8[:], swap)
            stot = pool.tile([8, 1], f32)
            nc.vector.tensor_add(stot[:], s8[:], shs[:])
            nls = pool.tile([8, 1], f32)
            nc.scalar.activation(nls[:], stot[:], AF.Ln)
            delta = pool.tile([8, 1], f16)
            nc.vector.tensor_sub(delta[:], negmC[:], nls[:])

            # ---- broadcast: psB[b] = row b of g (both halves) ----
            psBs = []
            for b in range(BS):
                psB = psum.tile([128, K], f32, tag=f"psb{b}", name=f"psb{b}")
                nc.tensor.matmul(
                    psB[:, 0:KH],
                    lhsT=sel8[:, b * 128 : (b + 1) * 128],
                    rhs=g8[:],
                    start=True,
                    stop=True,
                    skip_group_check=True,
                )
                nc.tensor.matmul(
                    psB[:, KH:K],
                    lhsT=sel8[:, (b + 4) * 128 : (b + 5) * 128],
                    rhs=g8[:],
                    start=True,
                    stop=True,
                    skip_group_check=True,
                )
                psBs.append(psB)
            psd = psum.tile([128, BS], f32)
            for b in range(BS):
                nc.tensor.matmul(
                    psd[:, b : b + 1],
                    lhsT=sel8[:, b * 128 : (b + 1) * 128],
                    rhs=delta[:],
                    start=True,
                    stop=True,
                    skip_group_check=True,
                )
            sL = pool.tile([128, BS], f32)
            nc.scalar.copy(sL[:], psd[:])

            # ---- normalize+cast copies + write (rows alternate DVE/ACT,
            # each packing two fp16 copies per partition -> 2 KB lines) ----
            for b in range(BS):
                psB = psBs[b]
                src2 = psB[:].unsqueeze(1).broadcast_to([128, 2, K])
                bt = pool.tile([128, 2, K], f16, tag=f"bt{b}", name=f"bt{b}")
                if b % 2 == 0:
                    nc.vector.tensor_scalar(
                        out=bt[:], in0=src2, scalar1=psd[:, b : b + 1],
                        scalar2=None, op0=ALU.add,
                    )
                else:
                    nc.scalar.activation(
                        bt[:], src2, AF.Identity, bias=sL[:, b : b + 1]
                    )
                nc.sync.dma_start(
                    out=out[b].rearrange("(p j u) k -> p j (u k)", j=RJ2, u=2),
                    in_=bt[:].rearrange("p u k -> p (u k)")
                    .unsqueeze(1)
                    .broadcast_to([128, RJ2, 2 * K]),
                )

    if split_waits:
        _split_multi_waits(nc, mybir)
    _BUILT[key] = nc
    return nc


def _split_multi_waits(nc, mybir):
    """This walrus build allows at most ONE sync wait per instruction.  Split
    any instruction with N>1 waits into N-1 single-wait NoOps on the same
    engine (executed immediately before it by the same sequencer) plus the
    original instruction carrying the final wait."""
    for fn in nc.m.functions:
        for blk in fn.blocks:
            new_insts = []
            for inst in blk.instructions:
                si = inst.sync_info
                if si is not None and len(si.on_wait) > 1:
                    waits = list(si.on_wait)
                    for i, w in enumerate(waits[:-1]):
                        new_insts.append(
                            mybir.InstNoOp(
                                name=f"{inst.name}-sw{i}",
                                engine=inst.engine,
                                sync_info=mybir.SyncInfo(
                                    on_wait=[w], on_update=[]
                                ),
                                bass_nofuse=True,
                            )
                        )
                    inst.sync_info = mybir.SyncInfo(
                        on_wait=[waits[-1]], on_update=list(si.on_update)
                    )
                new_insts.append(inst)
            blk.instructions = new_insts


def _run(inputs, trace=False, trace_kwargs=None):
    from concourse.bass_utils import run_bass_kernel_spmd

    nc = _build_nc()
    obvs = np.ascontiguousarray(np.asarray(inputs["obvs"], dtype=np.float32))
    mu_f = np.asarray(inputs["mu"], dtype=np.float32)
    ls_f = np.asarray(inputs["log_sigma"], dtype=np.float32)
    pi_f = np.asarray(inputs["ln_pi"], dtype=np.float32)
    di_f = np.asarray(inputs["ln_diag"], dtype=np.float32)
    rows = [
        mu_f,
        ls_f,
        -0.5 * (T + 1.0) * mu_f,
        2.0 * pi_f,
        (T - 1.0) * di_f,
        -(T + 1.0) * ls_f,
    ]
    params = {
        "prm16": np.ascontiguousarray(
            np.stack(
                [np.repeat(r.reshape(2, K // 2), B // NCORES, axis=0) for r in rows]
            ).astype(np.float16)
        )
    }
    params["c_misc"] = _const_misc()
    params["c_sel"] = _const_sel()
    in_maps = [
        {"obvs": obvs[c * BS : (c + 1) * BS], **params} for c in range(NCORES)
    ]
    kw = {}
    if trace:
        kw["trace"] = True
        if trace_kwargs:
            kw["trace_kwargs"] = trace_kwargs
    res = run_bass_kernel_spmd(nc, in_maps, list(range(NCORES)), **kw)
    full = np.empty((B, T, K), dtype=np.float32)
    for c in range(NCORES):
        full[c * BS : (c + 1) * BS] = np.asarray(
            res.results[c]["out"], dtype=np.float32
        )
    return full, res


def kernel(**inputs) -> np.ndarray:
    full, _ = _run(inputs, trace=False)
    return full

